# revision 17
# baseline (speedup 1.0000x reference)
"""MeanIoU kernel for Trainium2 (8 NeuronCores, Bass/Tile).

Problem: y_true, y_pred [8, 512, 512, 21] f32.
  t = argmax(y_true, -1); p = argmax(y_pred, -1)
  t_cnt[c] = #(t==c); p_cnt[c] = #(p==c); inter[c] = #(t==c & p==c)
  iou = inter / (t_cnt + p_cnt - inter); miou = mean over valid classes

Sharding: data-parallel, one batch image per core. Each core computes
per-class count partial sums; host all-reduces the tiny count vectors and
does the final division.

Per-core algorithm (image flattened to [128, 43008], partition p holds 2048
consecutive pixels x 21 classes):
  1. DMA tile [128, 5376] (256 pixels/partition)
  2. DVE tensor_reduce(max) over the innermost 21-class axis -> mx [128, 256]
  3. is_equal(x, mx broadcast) -> bf16 one-hot-ish mask (DVE)
  4. inter mask = eq_t * eq_p (DVE bf16 2x mode)
  5. PE: ones[128,1].T @ mask chunks [128, 336] accumulated in PSUM ->
     per-(pixel-subgroup, class) counts summed over partitions
  6. host: sum subgroups + cores, compute IoU

Measured (slope of in-NEFF repeats, 8 cores): ~120 us/run vs ~112 us
DMA-only floor (44 MB/core at ~390 GB/s). DVE is the compute-side critical
path; PE counting and the final division are fully hidden.

Exactness: counts match argmax semantics except for exact fp32 ties at the
max (probability ~1e-6 over the whole input; count error O(1) in ~100k).

Environment workarounds: the walrus here rejects >1 sync wait per
instruction (and >2 on CTRL) -> `_patched_drain_and_barrier` +
`_split_multi_waits` move excess waits onto single-wait NOPs. GPSIMD
elementwise (tensor_tensor on Pool) and InstPool are invalid ISA in this
toolchain, so everything vector-ish stays on DVE.
"""

import numpy as np
from contextlib import ExitStack

import concourse.bass as bass
import concourse.tile as tile
from concourse import mybir
from concourse.bass_utils import run_bass_kernel_spmd
from concourse.vector_clock import ScopedClock


def _patched_drain_and_barrier(self, tick_clock, wait_clock):
    # The stock Tile exit drain carries one sem wait per active logical proc;
    # the walrus in this container rejects CTRL instructions with >2 sync
    # waits ("Too many sync wait commands"). Split the waits across
    # single-wait NOPs instead.
    nc = self.nc
    probe = nc.sync.nop()
    wait_clock.add_sem_waits(
        probe.ins, ScopedClock({None: tick_clock.global_clock}))
    waits = list(probe.ins.sync_info.on_wait or [])
    if len(waits) > 1:
        probe.ins.sync_info.on_wait = [waits[0]]
        import bass_rust
        for w in waits[1:]:
            extra = nc.sync.nop()
            extra.ins.sync_info = bass_rust.SyncInfo(on_wait=[w],
                                                     on_update=[])
    nc.sync.drain()
    nc.all_engine_barrier()
    assert self.sems is not None
    popped = nc._tile_sem_poison_stack.pop()
    assert popped is self._sem_poison
    nc.clear_and_free_semaphores(list(self.sems.allocated().values()))
    nc.all_engine_barrier()


tile.TileContext._drain_and_barrier = _patched_drain_and_barrier


def _split_multi_waits(nc, limit=1):
    # Walrus in this container rejects instructions with more than one sync
    # wait (DMA) / two (CTRL). Split excess waits onto same-engine NOPs
    # inserted immediately before the instruction.
    import bass_rust
    k = 0
    for f in nc.m.functions:
        for b in f.blocks:
            insts = list(b.instructions)
            out, changed = [], False
            for inst in insts:
                si = inst.sync_info
                w = list(si.on_wait) if (si and si.on_wait) else []
                if len(w) > limit:
                    changed = True
                    n_extra = len(w) - limit
                    for ew in w[:n_extra]:
                        nop = mybir.InstNoOp(
                            name=f"I-waitsplit-{k}", ins=[], outs=[])
                        k += 1
                        nop.engine = inst.engine
                        nop.sync_info = bass_rust.SyncInfo(
                            on_wait=[ew], on_update=[])
                        nc.register_instruction(nop, overwrite=True)
                        out.append(nop)
                    si.on_wait = w[n_extra:]
                out.append(inst)
            if changed:
                b.instructions = out

P = 128                    # SBUF partitions
C = 21                     # classes
B = 8                      # batch (one image per core)
H = W = 512
NPIX = H * W               # pixels per core
PIX_PER_PART = NPIX // P   # 2048
FD = PIX_PER_PART * C      # 43008 f32 per partition per tensor
PIX_T = 256                # pixels per partition per tile
FD_T = PIX_T * C           # 5376
NT = PIX_PER_PART // PIX_T # 8 tiles per tensor
GRP = 16                   # pixel subgroups per matmul chunk
NCOL = GRP * C             # 336 columns per matmul (<=512)
NMM = FD_T // NCOL         # 16 matmuls per tile per mask
N_CORES = 8

# Dead knob: GPSIMD elementwise offload is rejected by this walrus
# ("Instruction engine check failed (Pool)"); kept only for API stability.
GP_PIX = 0


def build_nc(gp_pix: int = GP_PIX, repeats: int = 1,
             stages: str = "remi", use_pool_max: bool = False,
             in_bufs: int = 2, mask_bufs: int = 3, inter_bufs: int = 2,
             pix_t: int = PIX_T) -> bass.Bass:
    """stages: subset of 'r' (reduce), 'e' (eq), 'm' (matmuls), 'i' (inter).
    Dropping stages makes a timing-ablation variant (output garbage)."""
    nc = bass.Bass("TRN2", target_bir_lowering=False, debug=False,
                   num_devices=N_CORES)
    f32 = mybir.dt.float32
    xt = nc.dram_tensor("y_true", [P, FD], f32, kind="ExternalInput").ap()
    xp = nc.dram_tensor("y_pred", [P, FD], f32, kind="ExternalInput").ap()
    out = nc.dram_tensor("counts", [1, 3 * NCOL], f32,
                         kind="ExternalOutput").ap()
    with tile.TileContext(nc) as tc:
        _body(tc, out, xt, xp, gp_pix, repeats, stages, use_pool_max,
              in_bufs, mask_bufs, inter_bufs, pix_t)
    _split_multi_waits(nc)
    return nc


def _body(tc, out, xt, xp, gp_pix, repeats=1, stages="remi",
          use_pool_max=False, in_bufs=2, mask_bufs=2, inter_bufs=2,
          pix_t=PIX_T):
    nc = tc.nc
    f32 = mybir.dt.float32
    bf16 = mybir.dt.bfloat16
    PIX_T = pix_t
    FD_T = PIX_T * C
    NT = PIX_PER_PART // PIX_T
    NMM = FD_T // NCOL
    assert FD_T % NCOL == 0 and PIX_PER_PART % PIX_T == 0
    with ExitStack() as ctx:
        const_pool = ctx.enter_context(tc.tile_pool(name="const", bufs=1))
        in_pool = ctx.enter_context(tc.tile_pool(name="inp", bufs=in_bufs))
        mx_pool = ctx.enter_context(tc.tile_pool(name="mx", bufs=in_bufs))
        mask_pool = ctx.enter_context(
            tc.tile_pool(name="mask", bufs=mask_bufs))
        psum_pool = ctx.enter_context(
            tc.tile_pool(name="psum", bufs=1, space="PSUM"))

        ones = const_pool.tile([P, 1], bf16)
        nc.vector.memset(ones[:], 1.0)

        ps = [psum_pool.tile([1, NCOL], f32, tag=f"ps{m}", name=f"ps{m}")
              for m in range(3)]

        for _r in range(repeats):
          for i in range(NT):
            eqs = []
            for name, src in (("t", xt), ("p", xp)):
                xtile = in_pool.tile([P, FD_T], f32, tag=f"x{name}")
                nc.sync.dma_start(xtile[:], src[:, i * FD_T:(i + 1) * FD_T])
                x3 = xtile[:].rearrange("p (k c) -> p k c", c=C)
                mx = mx_pool.tile([P, PIX_T], f32, tag=f"m{name}")
                if "r" in stages:
                    if use_pool_max:
                        nc.vector.pool_max(mx[:], x3)
                    else:
                        nc.vector.tensor_reduce(
                            mx[:], x3, axis=mybir.AxisListType.X,
                            op=mybir.AluOpType.max)
                eq = mask_pool.tile([P, FD_T], bf16, tag=f"e{name}")
                if "e" in stages:
                    e3 = eq[:].rearrange("p (k c) -> p k c", c=C)
                    mxb = mx[:].unsqueeze(2).broadcast_to((P, PIX_T, C))
                    nc.vector.tensor_tensor(
                        e3, x3, mxb, op=mybir.AluOpType.is_equal)
                eqs.append(eq)
            masks = list(eqs)
            if "i" in stages:
                inter = mask_pool.tile([P, FD_T], bf16, tag="ei",
                                       bufs=inter_bufs)
                nc.vector.tensor_mul(inter[:], eqs[0][:], eqs[1][:])
                masks.append(inter)
            if "m" in stages:
                for m, msk in enumerate(masks):
                    for j in range(NMM):
                        nc.tensor.matmul(
                            ps[m][:], ones[:],
                            msk[:, j * NCOL:(j + 1) * NCOL],
                            start=(i == 0 and j == 0),
                            stop=(i == NT - 1 and j == NMM - 1))

        stage = const_pool.tile([1, 3 * NCOL], f32)
        for m in range(3):
            if "m" in stages:
                nc.scalar.copy(stage[:, m * NCOL:(m + 1) * NCOL], ps[m][:])
            else:
                nc.vector.memset(stage[:, m * NCOL:(m + 1) * NCOL], 0.0)
        nc.sync.dma_start(out[:], stage[:])


_NC_CACHE = None


def _get_nc():
    global _NC_CACHE
    if _NC_CACHE is None:
        _NC_CACHE = build_nc()
    return _NC_CACHE


def make_in_maps(y_true, y_pred):
    yt = np.ascontiguousarray(np.asarray(y_true, dtype=np.float32)).reshape(
        B, P, FD)
    yp = np.ascontiguousarray(np.asarray(y_pred, dtype=np.float32)).reshape(
        B, P, FD)
    return [{"y_true": yt[k], "y_pred": yp[k]} for k in range(N_CORES)]


def combine_counts(per_core_counts):
    """per_core_counts: iterable of [1, 3*NCOL] arrays -> miou scalar."""
    cnt = np.zeros((3, C), dtype=np.float64)
    for arr in per_core_counts:
        cnt += np.asarray(arr, dtype=np.float64).reshape(3, GRP, C).sum(axis=1)
    t_cnt, p_cnt, inter = cnt
    union = t_cnt + p_cnt - inter
    valid = union > 0
    iou = np.where(valid, inter / np.where(valid, union, 1.0), 0.0)
    return np.float32(iou.sum() / valid.sum())


def kernel(y_true, y_pred):
    nc = _get_nc()
    in_maps = make_in_maps(y_true, y_pred)
    res = run_bass_kernel_spmd(nc, in_maps, list(range(N_CORES)))
    return combine_counts([r["counts"] for r in res.results])
